# revision 41
# baseline (speedup 1.0000x reference)
"""Trainium2 Bass kernel for masked additive (Bahdanau-style) attention.

Computes, for each batch b:
    ph    = h_b @ U                     (T, H)
    e     = tanh(ph + s_b @ W) @ v      (T,)
    e     = where(mask, e, -1e9)
    score = softmax(e)                  (T,)
    ctx   = sum_t score_t * h_b[t]      (D,)

Key observations baked into the kernel:
  * The mask never needs to be applied to h: masked positions get energy
    -1e9, exp underflows to exactly 0 in fp32, so their contribution to
    the context is exactly 0 either way.
  * The big matmul (h @ U) is computed transposed: ph^T tiles with H on
    partitions, so the per-batch bias (s_b @ W) is a per-partition scalar
    that fuses into the tanh activation for free.
  * e is produced broadcast across all 128 partitions (the v-dot matmul
    uses a stationary operand whose 128 columns are all v), so the
    softmax runs at full 128-lane width with no partition reductions.
  * The softmax + context run flash-style per T-tile (local max/sum +
    fused multiply-accumulate over the resident h^T tile on the vector
    engine, rescaled at the end), so no h tile is ever touched twice, no
    serial softmax tail exists, and h needs no second load from HBM.
  * h^T and U are fed to the PE in bf16 (full-rate, half the HBM
    traffic); everything downstream of the big matmul accumulates in
    fp32. proj_s = s @ W stays in fp32r (full-rate fp32 storage).

Sharding: pure data parallelism, 4 batches per core on 8 cores; no
collectives. Host-side prep only shards and re-lays-out inputs
(transpose of h, bf16 casts).

Measured on trn2 (8 cores): ~322 us HW exec, scale-relative absmax
error ~4.6e-3 vs the fp32 reference.
"""

import ml_dtypes
import numpy as np

import concourse.bass as bass
import concourse.tile as tile
from concourse import bacc, mybir
from concourse.bass_utils import run_bass_kernel_spmd
from concourse.masks import make_identity

F32 = mybir.dt.float32
F32R = mybir.dt.float32r
BF16 = mybir.dt.bfloat16

B, T, D, H = 32, 2048, 1024, 1024
NCORES = 8
BL = B // NCORES          # batches per core
P = 128                   # partitions
KC = D // P               # 8 contraction chunks
MC = H // P               # 8 output-row chunks
TT = 512                  # T tile (fp32 moving-operand max, one PSUM bank)
NT = T // TT              # 4 T tiles per batch
AF = mybir.ActivationFunctionType
ALU = mybir.AluOpType


def _bcast_part(ap, parts=P):
    """Broadcast a 1-partition AP across `parts` partitions (step 0)."""
    return bass.AP(tensor=ap.tensor, offset=ap.offset, ap=[[0, parts]] + list(ap.ap))


def build_module():
    nc = bacc.Bacc(
        "TRN2",
        target_bir_lowering=False,
        debug=False,
        enable_asserts=False,
        num_devices=NCORES,
    )

    hT = nc.dram_tensor("hT", [BL, D, T], BF16, kind="ExternalInput").ap()
    sT = nc.dram_tensor("sT", [D, BL], F32R, kind="ExternalInput").ap()
    maskf = nc.dram_tensor("maskf", [BL, T], F32, kind="ExternalInput").ap()
    W = nc.dram_tensor("W", [D, H], F32R, kind="ExternalInput").ap()
    U = nc.dram_tensor("U", [D, H], BF16, kind="ExternalInput").ap()
    v = nc.dram_tensor("v", [H, 1], F32, kind="ExternalInput").ap()
    out = nc.dram_tensor("out", [BL, D], F32, kind="ExternalOutput").ap()

    with tile.TileContext(nc) as tc:
        with (
            tc.tile_pool(name="singles", bufs=1) as singles,
            tc.tile_pool(name="ht", bufs=5) as ht_pool,
            tc.tile_pool(name="mask", bufs=1) as mask_pool,
            tc.tile_pool(name="tanh", bufs=3) as tanh_pool,
            tc.tile_pool(name="p2", bufs=2) as p2_pool,
            tc.tile_pool(name="small", bufs=4) as small_pool,
            tc.tile_pool(name="ctx", bufs=2) as ctx_pool,
            tc.tile_pool(name="ps", bufs=5, space="PSUM") as ps_pool,
            tc.tile_pool(name="eps", bufs=3, space="PSUM") as e_pool,
        ):
            # ---- persistent operands -------------------------------------
            # DMA queue assignment (each issuing engine owns a DGE ring and
            # a ring drains in issue order, so placement controls priority):
            # gpsimd carries sT + W first, then all hT; scalar carries U and
            # the outputs; sync carries v + the mask broadcasts.
            sT_sb = singles.tile([P, KC, BL], F32R)
            nc.gpsimd.dma_start(
                out=sT_sb, in_=sT.rearrange("(kc p) b -> p kc b", p=P)
            )

            # W fully resident in kc-pair chunks, issued FIRST on the gpsimd
            # queue (ahead of all hT traffic): queues drain in order, so W —
            # which gates proj -> tanh -> everything — wins the early DMA
            # engines. (Streaming W in small WAR-chained chunks starved
            # behind hT in DMA-engine arbitration — measured 65us stall.)
            w_sb = singles.tile([P, KC, H], F32R)
            W_r = W.rearrange("(kc p) n -> p kc n", p=P)
            for wc in range(4):
                nc.gpsimd.dma_start(
                    out=w_sb[:, 2 * wc : 2 * wc + 2, :],
                    in_=W_r[:, 2 * wc : 2 * wc + 2, :],
                )

            # U chunked by H-column pairs: the first main matmul group only
            # needs U[:, :, :128], so it starts ~3 chunks earlier.
            u_sb = singles.tile([P, KC, H], BF16)
            U_r = U.rearrange("(kc p) n -> p kc n", p=P)
            for uc in range(4):
                nc.scalar.dma_start(
                    out=u_sb[:, :, uc * 256 : (uc + 1) * 256],
                    in_=U_r[:, :, uc * 256 : (uc + 1) * 256],
                )

            # v replicated into a (P, MC, P) stationary operand: for each
            # H-chunk mc, all 128 columns equal v[mc*128 + p].
            v_col = singles.tile([P, MC], F32)
            nc.sync.dma_start(out=v_col, in_=v.rearrange("(mc p) x -> p (mc x)", p=P))
            v_bc = singles.tile([P, MC, P], BF16)
            for mc in range(MC):
                nc.vector.memset(v_bc[:, mc, :], 0.0)
                nc.vector.tensor_scalar_add(
                    out=v_bc[:, mc, :],
                    in0=v_bc[:, mc, :],
                    scalar1=v_col[:, mc : mc + 1],
                )

            # ---- phase 0: proj_s = s @ W, then on-chip transpose ----------
            # sT-stationary (the weight load is only BL=4 columns, vs 64
            # full 128-column fp32 loads the W-stationary form needs — that
            # cost ~29us of PE time), then 16 PE transposes of (4,128)
            # chunks put H on partitions for the tanh bias.
            identity4 = singles.tile([BL, BL], F32)
            make_identity(nc, identity4)
            pnat = [
                ps_pool.tile([BL, TT], F32, tag="ps", name=f"pnat{i}")
                for i in range(2)
            ]
            for kc in range(KC):
                for nh in range(2):
                    nc.tensor.matmul(
                        pnat[nh],
                        lhsT=sT_sb[:, kc, :],
                        rhs=w_sb[:, kc, nh * TT : (nh + 1) * TT],
                        start=(kc == 0),
                        stop=(kc == KC - 1),
                    )
            pstg = singles.tile([BL, H], F32)
            for nh in range(2):
                nc.vector.tensor_copy(
                    out=pstg[:, nh * TT : (nh + 1) * TT], in_=pnat[nh]
                )
            proj_sb = singles.tile([P, MC, BL], F32)
            for mc in range(MC):
                tp = ps_pool.tile([P, BL], F32, tag="ps", name=f"tp{mc}")
                nc.tensor.transpose(
                    tp, in_=pstg[:, mc * P : (mc + 1) * P], identity=identity4
                )
                nc.vector.tensor_copy(out=proj_sb[:, mc, :], in_=tp)

            # ---- per-batch main pipeline ---------------------------------
            for b in range(BL):
                # per-T-tile hT tiles: with the online context accumulation
                # each tile is dead after its own T-tile pass, so no
                # full-batch residency is needed and bufs=5 gives prefetch.
                hT_b = hT[b].rearrange("(kc p) t -> p kc t", p=P)
                ht_tiles = []
                for tt in range(NT):
                    htt = ht_pool.tile([P, KC, TT], BF16, tag="ht")
                    nc.gpsimd.dma_start(
                        out=htt, in_=hT_b[:, :, tt * TT : (tt + 1) * TT]
                    )
                    ht_tiles.append(htt)
                mb_sb = mask_pool.tile([P, T], F32, tag="m")
                nc.sync.dma_start(out=mb_sb, in_=_bcast_part(maskf[b]))

                # phase 1 with online (flash-style) softmax + context:
                # per T-tile, right after its energies e_ps complete:
                #   et  = (e + 512) * m     (masked -> 0; 512 > max|e| and
                #         exp(-512-max) underflows to exactly 0 in fp32,
                #         while ulp(512)=6.1e-5 keeps e's precision)
                #   nmax_i = -max(et); ex = exp(et - max_i); z_i = sum(ex)
                #   part[:, dc, i] = sum_t ex_t * hT[p, dc, t]
                # tiny per-batch tail: f_i = exp(max_i - M) with global max M,
                # ctx = sum_i part_i f_i / sum_i z_i f_i.
                nmax = small_pool.tile([P, NT], F32, tag="nmax")
                zs = small_pool.tile([P, NT], F32, tag="zs")
                part = ctx_pool.tile([P, KC, NT], F32, tag="part")
                scr = p2_pool.tile([P, TT], F32, tag="scr")
                for tt in range(NT):
                    e_ps = e_pool.tile([P, TT], F32, tag="e")
                    for mc in range(MC):
                        pp = ps_pool.tile([P, TT], F32, tag="ps")
                        for kc in range(KC):
                            nc.tensor.matmul(
                                pp,
                                lhsT=u_sb[:, kc, mc * P : (mc + 1) * P],
                                rhs=ht_tiles[tt][:, kc, :],
                                start=(kc == 0),
                                stop=(kc == KC - 1),
                            )
                        th = tanh_pool.tile([P, TT], BF16, tag="th")
                        nc.scalar.activation(
                            out=th,
                            in_=pp,
                            func=AF.Tanh,
                            bias=proj_sb[:, mc, b : b + 1],
                            scale=1.0,
                        )
                        nc.tensor.matmul(
                            e_ps,
                            lhsT=v_bc[:, mc, :],
                            rhs=th,
                            start=(mc == 0),
                            stop=(mc == MC - 1),
                        )
                    et = p2_pool.tile([P, TT], F32, tag="et")
                    nc.vector.scalar_tensor_tensor(
                        out=et,
                        in0=e_ps,
                        scalar=512.0,
                        in1=mb_sb[:, tt * TT : (tt + 1) * TT],
                        op0=ALU.add,
                        op1=ALU.mult,
                    )
                    nc.vector.tensor_reduce(
                        out=nmax[:, tt : tt + 1],
                        in_=et,
                        axis=mybir.AxisListType.X,
                        op=ALU.max,
                        negate=True,
                    )
                    ex = p2_pool.tile([P, TT], F32, tag="ex")
                    nc.scalar.activation(
                        out=ex,
                        in_=et,
                        func=AF.Exp,
                        bias=nmax[:, tt : tt + 1],
                        scale=1.0,
                        accum_out=zs[:, tt : tt + 1],
                    )
                    for dc in range(KC):
                        nc.vector.scalar_tensor_tensor(
                            out=scr,
                            in0=ht_tiles[tt][:, dc, :],
                            scalar=1.0,
                            in1=ex,
                            op0=ALU.mult,
                            op1=ALU.mult,
                            accum_out=part[:, dc, tt : tt + 1],
                        )

                # per-batch tail (all on (P, NT)/(P, KC) tiles — tiny)
                negM = small_pool.tile([P, 1], F32, tag="negM")
                nc.vector.tensor_reduce(
                    out=negM, in_=nmax, axis=mybir.AxisListType.X, op=ALU.min
                )
                f = small_pool.tile([P, NT], F32, tag="f")
                nc.scalar.activation(
                    out=f, in_=nmax, func=AF.Exp, bias=negM, scale=-1.0
                )
                fz = small_pool.tile([P, NT], F32, tag="fz")
                zf = small_pool.tile([P, 1], F32, tag="zf")
                nc.vector.scalar_tensor_tensor(
                    out=fz,
                    in0=zs,
                    scalar=1.0,
                    in1=f,
                    op0=ALU.mult,
                    op1=ALU.mult,
                    accum_out=zf,
                )
                sinv = small_pool.tile([P, 1], F32, tag="sinv")
                nc.vector.reciprocal(sinv, zf)
                for tt in range(NT):
                    nc.vector.tensor_scalar_mul(
                        out=part[:, :, tt : tt + 1],
                        in0=part[:, :, tt : tt + 1],
                        scalar1=f[:, tt : tt + 1],
                    )
                ctx = ctx_pool.tile([P, KC], F32, tag="ctx")
                nc.vector.tensor_reduce(
                    out=ctx, in_=part, axis=mybir.AxisListType.X, op=ALU.add
                )
                nc.vector.tensor_scalar_mul(out=ctx, in0=ctx, scalar1=sinv)
                nc.scalar.dma_start(
                    out=out[b].rearrange("(dc p) -> p dc", p=P), in_=ctx
                )

    nc.compile()
    return nc


_NC_CACHE = None


def _get_module():
    global _NC_CACHE
    if _NC_CACHE is None:
        _NC_CACHE = build_module()
    return _NC_CACHE


def core_in_map(s, h, mask, W, U, v, c):
    """Shard + lay out the full inputs for core c."""
    bs = slice(c * BL, (c + 1) * BL)
    return {
        "hT": np.ascontiguousarray(
            np.asarray(h, np.float32)[bs]
            .transpose(0, 2, 1)
            .astype(ml_dtypes.bfloat16)
        ),
        "sT": np.ascontiguousarray(np.asarray(s, np.float32)[0, bs].T),
        "maskf": np.ascontiguousarray(np.asarray(mask)[bs].astype(np.float32)),
        "W": np.ascontiguousarray(np.asarray(W, np.float32)),
        "U": np.ascontiguousarray(np.asarray(U, np.float32).astype(ml_dtypes.bfloat16)),
        "v": np.ascontiguousarray(np.asarray(v, np.float32).reshape(H, 1)),
    }


def kernel(s, h, mask, W, U, v):
    in_maps = [core_in_map(s, h, mask, W, U, v, c) for c in range(NCORES)]
    nc = _get_module()
    res = run_bass_kernel_spmd(nc, in_maps, list(range(NCORES)))
    return np.concatenate([res.results[c]["out"] for c in range(NCORES)], axis=0)


# revision 43
# speedup vs baseline: 1.0470x; 1.0470x over previous
"""Trainium2 Bass kernel for masked additive (Bahdanau-style) attention.

Computes, for each batch b:
    ph    = h_b @ U                     (T, H)
    e     = tanh(ph + s_b @ W) @ v      (T,)
    e     = where(mask, e, -1e9)
    score = softmax(e)                  (T,)
    ctx   = sum_t score_t * h_b[t]      (D,)

Key observations baked into the kernel:
  * The mask never needs to be applied to h: masked positions get energy
    -1e9, exp underflows to exactly 0 in fp32, so their contribution to
    the context is exactly 0 either way.
  * The big matmul (h @ U) is computed transposed: ph^T tiles with H on
    partitions, so the per-batch bias (s_b @ W) is a per-partition scalar
    that fuses into the tanh activation for free.
  * e is produced broadcast across all 128 partitions (the v-dot matmul
    uses a stationary operand whose 128 columns are all v), so the
    softmax runs at full 128-lane width with no partition reductions.
  * The softmax + context run flash-style per T-tile (local max/sum +
    fused multiply-accumulate over the resident h^T tile on the vector
    engine, rescaled at the end), so no h tile is ever touched twice, no
    serial softmax tail exists, and h needs no second load from HBM.
  * h^T and U are fed to the PE in bf16 (full-rate, half the HBM
    traffic); everything downstream of the big matmul accumulates in
    fp32. proj_s = s @ W stays in fp32r (full-rate fp32 storage).

Sharding: pure data parallelism, 4 batches per core on 8 cores; no
collectives. Host-side prep only shards and re-lays-out inputs
(transpose of h, bf16 casts).

Measured on trn2 (8 cores): ~322 us HW exec, scale-relative absmax
error ~4.6e-3 vs the fp32 reference.
"""

import ml_dtypes
import numpy as np

import concourse.bass as bass
import concourse.tile as tile
from concourse import bacc, mybir
from concourse.bass_utils import run_bass_kernel_spmd
from concourse.masks import make_identity

F32 = mybir.dt.float32
F32R = mybir.dt.float32r
BF16 = mybir.dt.bfloat16

B, T, D, H = 32, 2048, 1024, 1024
NCORES = 8
BL = B // NCORES          # batches per core
P = 128                   # partitions
KC = D // P               # 8 contraction chunks
MC = H // P               # 8 output-row chunks
TT = 512                  # T tile (fp32 moving-operand max, one PSUM bank)
NT = T // TT              # 4 T tiles per batch
AF = mybir.ActivationFunctionType
ALU = mybir.AluOpType


def _bcast_part(ap, parts=P):
    """Broadcast a 1-partition AP across `parts` partitions (step 0)."""
    return bass.AP(tensor=ap.tensor, offset=ap.offset, ap=[[0, parts]] + list(ap.ap))


def build_module():
    nc = bacc.Bacc(
        "TRN2",
        target_bir_lowering=False,
        debug=False,
        enable_asserts=False,
        num_devices=NCORES,
    )

    hT = nc.dram_tensor("hT", [BL, D, T], BF16, kind="ExternalInput").ap()
    sT = nc.dram_tensor("sT", [D, BL], F32R, kind="ExternalInput").ap()
    maskf = nc.dram_tensor("maskf", [BL, T], F32, kind="ExternalInput").ap()
    W = nc.dram_tensor("W", [D, H], F32R, kind="ExternalInput").ap()
    U = nc.dram_tensor("U", [D, H], BF16, kind="ExternalInput").ap()
    v = nc.dram_tensor("v", [H, 1], F32, kind="ExternalInput").ap()
    out = nc.dram_tensor("out", [BL, D], F32, kind="ExternalOutput").ap()

    with tile.TileContext(nc) as tc:
        with (
            tc.tile_pool(name="singles", bufs=1) as singles,
            tc.tile_pool(name="ht", bufs=5) as ht_pool,
            tc.tile_pool(name="mask", bufs=1) as mask_pool,
            tc.tile_pool(name="tanh", bufs=3) as tanh_pool,
            tc.tile_pool(name="p2", bufs=2) as p2_pool,
            tc.tile_pool(name="small", bufs=4) as small_pool,
            tc.tile_pool(name="ctx", bufs=2) as ctx_pool,
            tc.tile_pool(name="ps", bufs=5, space="PSUM") as ps_pool,
            tc.tile_pool(name="eps", bufs=3, space="PSUM") as e_pool,
        ):
            # ---- persistent operands -------------------------------------
            # DMA queue assignment (each issuing engine owns a DGE ring and
            # a ring drains in issue order, so placement controls priority):
            # gpsimd carries sT + W first, then all hT; scalar carries U and
            # the outputs; sync carries v + the mask broadcasts.
            sT_sb = singles.tile([P, KC, BL], F32R)
            nc.gpsimd.dma_start(
                out=sT_sb, in_=sT.rearrange("(kc p) b -> p kc b", p=P)
            )

            # W fully resident in kc-pair chunks, issued FIRST on the gpsimd
            # queue (ahead of all hT traffic): queues drain in order, so W —
            # which gates proj -> tanh -> everything — wins the early DMA
            # engines. (Streaming W in small WAR-chained chunks starved
            # behind hT in DMA-engine arbitration — measured 65us stall.)
            w_sb = singles.tile([P, KC, H], F32R)
            W_r = W.rearrange("(kc p) n -> p kc n", p=P)
            for wc in range(4):
                nc.gpsimd.dma_start(
                    out=w_sb[:, 2 * wc : 2 * wc + 2, :],
                    in_=W_r[:, 2 * wc : 2 * wc + 2, :],
                )

            # U chunked by H-column pairs: the first main matmul group only
            # needs U[:, :, :128], so it starts ~3 chunks earlier.
            u_sb = singles.tile([P, KC, H], BF16)
            U_r = U.rearrange("(kc p) n -> p kc n", p=P)
            for uc in range(4):
                nc.scalar.dma_start(
                    out=u_sb[:, :, uc * 256 : (uc + 1) * 256],
                    in_=U_r[:, :, uc * 256 : (uc + 1) * 256],
                )

            # v replicated into a (P, MC, P) stationary operand: for each
            # H-chunk mc, all 128 columns equal v[mc*128 + p].
            v_col = singles.tile([P, MC], F32)
            nc.sync.dma_start(out=v_col, in_=v.rearrange("(mc p) x -> p (mc x)", p=P))
            v_bc = singles.tile([P, MC, P], BF16)
            for mc in range(MC):
                nc.vector.memset(v_bc[:, mc, :], 0.0)
                nc.vector.tensor_scalar_add(
                    out=v_bc[:, mc, :],
                    in0=v_bc[:, mc, :],
                    scalar1=v_col[:, mc : mc + 1],
                )

            # ---- emission helpers -----------------------------------------
            # Tile's scheduler orders per-engine streams by dependency +
            # emission priority, so emission order biases what the PE does
            # while waiting on DMA.

            def emit_batch_dmas(b, pre_tt0=None):
                hT_b = hT[b].rearrange("(kc p) t -> p kc t", p=P)
                ht_tiles = []
                for tt in range(NT):
                    if tt == 0 and pre_tt0 is not None:
                        ht_tiles.append(pre_tt0)
                        continue
                    htt = ht_pool.tile(
                        [P, KC, TT], BF16, tag="ht", name=f"ht_b{b}t{tt}"
                    )
                    nc.gpsimd.dma_start(
                        out=htt, in_=hT_b[:, :, tt * TT : (tt + 1) * TT]
                    )
                    ht_tiles.append(htt)
                mb_sb = mask_pool.tile([P, T], F32, tag="m", name=f"mb{b}")
                nc.sync.dma_start(out=mb_sb, in_=_bcast_part(maskf[b]))
                return ht_tiles, mb_sb

            def emit_mains(b, tt, ht_tiles):
                pps = []
                for mc in range(MC):
                    pp = ps_pool.tile(
                        [P, TT], F32, tag="ps", name=f"pp{b}_{tt}_{mc}"
                    )
                    for kc in range(KC):
                        nc.tensor.matmul(
                            pp,
                            lhsT=u_sb[:, kc, mc * P : (mc + 1) * P],
                            rhs=ht_tiles[tt][:, kc, :],
                            start=(kc == 0),
                            stop=(kc == KC - 1),
                        )
                    pps.append(pp)
                return pps

            def emit_tile_rest(b, tt, pps, ht_tiles, mb_sb, st):
                # tanh + v-dot, then the online-softmax tile pass:
                #   et  = (e + 512) * m   (masked -> 0; 512 > max|e| and
                #         exp(-512-max) underflows to exactly 0 in fp32,
                #         while ulp(512)=6.1e-5 keeps e's precision)
                #   nmax_i = -max(et); ex = exp(et - max_i); z_i = sum(ex)
                #   part[:, dc, i] = sum_t ex_t * hT[p, dc, t]
                nmax, zs, part, scr = st
                e_ps = e_pool.tile([P, TT], F32, tag="e", name=f"e{b}_{tt}")
                for mc in range(MC):
                    th = tanh_pool.tile(
                        [P, TT], BF16, tag="th", name=f"th{b}_{tt}_{mc}"
                    )
                    nc.scalar.activation(
                        out=th,
                        in_=pps[mc],
                        func=AF.Tanh,
                        bias=proj_sb[:, mc, b : b + 1],
                        scale=1.0,
                    )
                    nc.tensor.matmul(
                        e_ps,
                        lhsT=v_bc[:, mc, :],
                        rhs=th,
                        start=(mc == 0),
                        stop=(mc == MC - 1),
                    )
                et = p2_pool.tile([P, TT], F32, tag="et", name=f"et{b}_{tt}")
                nc.vector.scalar_tensor_tensor(
                    out=et,
                    in0=e_ps,
                    scalar=512.0,
                    in1=mb_sb[:, tt * TT : (tt + 1) * TT],
                    op0=ALU.add,
                    op1=ALU.mult,
                )
                nc.vector.tensor_reduce(
                    out=nmax[:, tt : tt + 1],
                    in_=et,
                    axis=mybir.AxisListType.X,
                    op=ALU.max,
                    negate=True,
                )
                ex = p2_pool.tile([P, TT], F32, tag="ex", name=f"ex{b}_{tt}")
                nc.scalar.activation(
                    out=ex,
                    in_=et,
                    func=AF.Exp,
                    bias=nmax[:, tt : tt + 1],
                    scale=1.0,
                    accum_out=zs[:, tt : tt + 1],
                )
                for dc in range(KC):
                    nc.vector.scalar_tensor_tensor(
                        out=scr,
                        in0=ht_tiles[tt][:, dc, :],
                        scalar=1.0,
                        in1=ex,
                        op0=ALU.mult,
                        op1=ALU.mult,
                        accum_out=part[:, dc, tt : tt + 1],
                    )

            def emit_batch_tail(b, st):
                # combine tiles: f_i = exp(max_i - M) with global max M,
                # ctx = sum_i part_i f_i / sum_i z_i f_i  (all tiny tiles)
                nmax, zs, part, scr = st
                negM = small_pool.tile([P, 1], F32, tag="negM", name=f"nM{b}")
                nc.vector.tensor_reduce(
                    out=negM, in_=nmax, axis=mybir.AxisListType.X, op=ALU.min
                )
                f = small_pool.tile([P, NT], F32, tag="f", name=f"f{b}")
                nc.scalar.activation(
                    out=f, in_=nmax, func=AF.Exp, bias=negM, scale=-1.0
                )
                fz = small_pool.tile([P, NT], F32, tag="fz", name=f"fz{b}")
                zf = small_pool.tile([P, 1], F32, tag="zf", name=f"zf{b}")
                nc.vector.scalar_tensor_tensor(
                    out=fz,
                    in0=zs,
                    scalar=1.0,
                    in1=f,
                    op0=ALU.mult,
                    op1=ALU.mult,
                    accum_out=zf,
                )
                sinv = small_pool.tile([P, 1], F32, tag="sinv", name=f"si{b}")
                nc.vector.reciprocal(sinv, zf)
                for tt in range(NT):
                    nc.vector.tensor_scalar_mul(
                        out=part[:, :, tt : tt + 1],
                        in0=part[:, :, tt : tt + 1],
                        scalar1=f[:, tt : tt + 1],
                    )
                ctx = ctx_pool.tile([P, KC], F32, tag="ctx", name=f"cx{b}")
                nc.vector.tensor_reduce(
                    out=ctx, in_=part, axis=mybir.AxisListType.X, op=ALU.add
                )
                nc.vector.tensor_scalar_mul(out=ctx, in0=ctx, scalar1=sinv)
                nc.scalar.dma_start(
                    out=out[b].rearrange("(dc p) -> p dc", p=P), in_=ctx
                )

            def batch_state(b):
                nmax = small_pool.tile([P, NT], F32, tag="nmax", name=f"nm{b}")
                zs = small_pool.tile([P, NT], F32, tag="zs", name=f"zs{b}")
                part = ctx_pool.tile([P, KC, NT], F32, tag="part", name=f"pt{b}")
                scr = p2_pool.tile([P, TT], F32, tag="scr", name=f"sc{b}")
                return nmax, zs, part, scr

            def emit_proj():
                # proj_s = s @ W (sT-stationary: the weight load is only
                # BL=4 columns), then 16 PE transposes of (4,128) chunks put
                # H on partitions for the tanh bias. No DRAM round-trip —
                # its tiny-line descriptors clogged a DMA queue for ~50us.
                pnat = []
                for i in range(2):
                    pn = e_pool.tile([BL, TT], F32, tag="e", name=f"pnat{i}")
                    pnat.append(pn)
                for kc in range(KC):
                    for nh in range(2):
                        nc.tensor.matmul(
                            pnat[nh],
                            lhsT=sT_sb[:, kc, :],
                            rhs=w_sb[:, kc, nh * TT : (nh + 1) * TT],
                            start=(kc == 0),
                            stop=(kc == KC - 1),
                        )
                pstg = singles.tile([BL, H], F32)
                for nh in range(2):
                    nc.vector.tensor_copy(
                        out=pstg[:, nh * TT : (nh + 1) * TT], in_=pnat[nh]
                    )
                proj_sb = singles.tile([P, MC, BL], F32)
                for mc in range(MC):
                    tp = e_pool.tile([P, BL], F32, tag="e", name=f"tp{mc}")
                    nc.tensor.transpose(
                        tp, in_=pstg[:, mc * P : (mc + 1) * P], identity=identity4
                    )
                    nc.vector.tensor_copy(out=proj_sb[:, mc, :], in_=tp)
                return proj_sb

            identity4 = singles.tile([BL, BL], F32)
            make_identity(nc, identity4)

            # ---- pipeline -------------------------------------------------
            # Batch 0, tile 0's main matmuls are emitted BEFORE proj: they
            # only need hT(b0,tt0) + the first U chunk, both of which land
            # well before all of W, so the PE warms up on dense main work
            # while W trickles in; the scheduler slots proj into the psum-
            # runway stall that follows.
            ht0, mb0 = emit_batch_dmas(0)
            st0 = batch_state(0)
            pps00 = emit_mains(0, 0, ht0)
            proj_sb = emit_proj()
            emit_tile_rest(0, 0, pps00, ht0, mb0, st0)
            for tt in range(1, NT):
                pps = emit_mains(0, tt, ht0)
                emit_tile_rest(0, tt, pps, ht0, mb0, st0)
            emit_batch_tail(0, st0)

            for b in range(1, BL):
                ht_tiles, mb_sb = emit_batch_dmas(b)
                st = batch_state(b)
                for tt in range(NT):
                    pps = emit_mains(b, tt, ht_tiles)
                    emit_tile_rest(b, tt, pps, ht_tiles, mb_sb, st)
                emit_batch_tail(b, st)

    nc.compile()
    return nc


_NC_CACHE = None


def _get_module():
    global _NC_CACHE
    if _NC_CACHE is None:
        _NC_CACHE = build_module()
    return _NC_CACHE


def core_in_map(s, h, mask, W, U, v, c):
    """Shard + lay out the full inputs for core c."""
    bs = slice(c * BL, (c + 1) * BL)
    return {
        "hT": np.ascontiguousarray(
            np.asarray(h, np.float32)[bs]
            .transpose(0, 2, 1)
            .astype(ml_dtypes.bfloat16)
        ),
        "sT": np.ascontiguousarray(np.asarray(s, np.float32)[0, bs].T),
        "maskf": np.ascontiguousarray(np.asarray(mask)[bs].astype(np.float32)),
        "W": np.ascontiguousarray(np.asarray(W, np.float32)),
        "U": np.ascontiguousarray(np.asarray(U, np.float32).astype(ml_dtypes.bfloat16)),
        "v": np.ascontiguousarray(np.asarray(v, np.float32).reshape(H, 1)),
    }


def kernel(s, h, mask, W, U, v):
    in_maps = [core_in_map(s, h, mask, W, U, v, c) for c in range(NCORES)]
    nc = _get_module()
    res = run_bass_kernel_spmd(nc, in_maps, list(range(NCORES)))
    return np.concatenate([res.results[c]["out"] for c in range(NCORES)], axis=0)


# revision 45
# speedup vs baseline: 1.0482x; 1.0011x over previous
"""Trainium2 Bass kernel for masked additive (Bahdanau-style) attention.

Computes, for each batch b:
    ph    = h_b @ U                     (T, H)
    e     = tanh(ph + s_b @ W) @ v      (T,)
    e     = where(mask, e, -1e9)
    score = softmax(e)                  (T,)
    ctx   = sum_t score_t * h_b[t]      (D,)

Key observations baked into the kernel:
  * The mask never needs to be applied to h: masked positions get energy
    -1e9, exp underflows to exactly 0 in fp32, so their contribution to
    the context is exactly 0 either way.
  * The big matmul (h @ U) is computed transposed: ph^T tiles with H on
    partitions, so the per-batch bias (s_b @ W) is a per-partition scalar
    that fuses into the tanh activation for free.
  * e is produced broadcast across all 128 partitions (the v-dot matmul
    uses a stationary operand whose 128 columns are all v), so the
    softmax runs at full 128-lane width with no partition reductions.
  * The softmax + context run flash-style per T-tile (local max/sum +
    fused multiply-accumulate over the resident h^T tile on the vector
    engine, rescaled at the end), so no h tile is ever touched twice, no
    serial softmax tail exists, and h needs no second load from HBM.
  * h^T and U are fed to the PE in bf16 (full-rate, half the HBM
    traffic); everything downstream of the big matmul accumulates in
    fp32. proj_s = s @ W stays in fp32r (full-rate fp32 storage).

Sharding: pure data parallelism, 4 batches per core on 8 cores; no
collectives. Host-side prep only shards and re-lays-out inputs
(transpose of h, bf16 casts).

Measured on trn2 (8 cores): ~322 us HW exec, scale-relative absmax
error ~4.6e-3 vs the fp32 reference.
"""

import ml_dtypes
import numpy as np

import concourse.bass as bass
import concourse.tile as tile
from concourse import bacc, mybir
from concourse.bass_utils import run_bass_kernel_spmd
from concourse.masks import make_identity

F32 = mybir.dt.float32
F32R = mybir.dt.float32r
BF16 = mybir.dt.bfloat16

B, T, D, H = 32, 2048, 1024, 1024
NCORES = 8
BL = B // NCORES          # batches per core
P = 128                   # partitions
KC = D // P               # 8 contraction chunks
MC = H // P               # 8 output-row chunks
TT = 512                  # T tile (fp32 moving-operand max, one PSUM bank)
NT = T // TT              # 4 T tiles per batch
AF = mybir.ActivationFunctionType
ALU = mybir.AluOpType


def _bcast_part(ap, parts=P):
    """Broadcast a 1-partition AP across `parts` partitions (step 0)."""
    return bass.AP(tensor=ap.tensor, offset=ap.offset, ap=[[0, parts]] + list(ap.ap))


def build_module():
    nc = bacc.Bacc(
        "TRN2",
        target_bir_lowering=False,
        debug=False,
        enable_asserts=False,
        num_devices=NCORES,
    )

    hT = nc.dram_tensor("hT", [BL, D, T], BF16, kind="ExternalInput").ap()
    sT = nc.dram_tensor("sT", [D, BL], F32R, kind="ExternalInput").ap()
    maskf = nc.dram_tensor("maskf", [BL, T], F32, kind="ExternalInput").ap()
    W = nc.dram_tensor("W", [D, H], F32R, kind="ExternalInput").ap()
    U = nc.dram_tensor("U", [D, H], BF16, kind="ExternalInput").ap()
    v = nc.dram_tensor("v", [H, 1], F32, kind="ExternalInput").ap()
    out = nc.dram_tensor("out", [BL, D], F32, kind="ExternalOutput").ap()

    with tile.TileContext(nc) as tc:
        with (
            tc.tile_pool(name="singles", bufs=1) as singles,
            tc.tile_pool(name="ht", bufs=5) as ht_pool,
            tc.tile_pool(name="mask", bufs=1) as mask_pool,
            tc.tile_pool(name="tanh", bufs=3) as tanh_pool,
            tc.tile_pool(name="p2", bufs=2) as p2_pool,
            tc.tile_pool(name="small", bufs=4) as small_pool,
            tc.tile_pool(name="ctx", bufs=2) as ctx_pool,
            tc.tile_pool(name="ps", bufs=5, space="PSUM") as ps_pool,
            tc.tile_pool(name="eps", bufs=3, space="PSUM") as e_pool,
        ):
            # ---- persistent operands -------------------------------------
            # DMA queue assignment (each issuing engine owns a DGE ring and
            # a ring drains in issue order, so placement controls priority):
            # gpsimd carries sT + W first, then all hT; scalar carries U and
            # the outputs; sync carries v + the mask broadcasts.
            sT_sb = singles.tile([P, KC, BL], F32R)
            nc.gpsimd.dma_start(
                out=sT_sb, in_=sT.rearrange("(kc p) b -> p kc b", p=P)
            )

            # W fully resident in kc-pair chunks, issued FIRST on the gpsimd
            # queue (ahead of all hT traffic): queues drain in order, so W —
            # which gates proj -> tanh -> everything — wins the early DMA
            # engines. (Streaming W in small WAR-chained chunks starved
            # behind hT in DMA-engine arbitration — measured 65us stall.)
            # batch 0 tile 0's hT goes ahead of even W: it unblocks the
            # first 5 main-matmul groups, which is what the PE runs while
            # W arrives for proj.
            ht00 = ht_pool.tile([P, KC, TT], BF16, tag="ht", name="ht_b0t0")
            nc.gpsimd.dma_start(
                out=ht00,
                in_=hT[0].rearrange("(kc p) t -> p kc t", p=P)[:, :, 0:TT],
            )

            w_sb = singles.tile([P, KC, H], F32R)
            W_r = W.rearrange("(kc p) n -> p kc n", p=P)
            for wc in range(4):
                nc.gpsimd.dma_start(
                    out=w_sb[:, 2 * wc : 2 * wc + 2, :],
                    in_=W_r[:, 2 * wc : 2 * wc + 2, :],
                )

            # U chunked by H-column pairs: the first main matmul group only
            # needs U[:, :, :128], so it starts ~3 chunks earlier.
            u_sb = singles.tile([P, KC, H], BF16)
            U_r = U.rearrange("(kc p) n -> p kc n", p=P)
            for uc in range(4):
                nc.scalar.dma_start(
                    out=u_sb[:, :, uc * 256 : (uc + 1) * 256],
                    in_=U_r[:, :, uc * 256 : (uc + 1) * 256],
                )

            # v replicated into a (P, MC, P) stationary operand: for each
            # H-chunk mc, all 128 columns equal v[mc*128 + p].
            v_col = singles.tile([P, MC], F32)
            nc.sync.dma_start(out=v_col, in_=v.rearrange("(mc p) x -> p (mc x)", p=P))
            v_bc = singles.tile([P, MC, P], BF16)
            for mc in range(MC):
                nc.vector.memset(v_bc[:, mc, :], 0.0)
                nc.vector.tensor_scalar_add(
                    out=v_bc[:, mc, :],
                    in0=v_bc[:, mc, :],
                    scalar1=v_col[:, mc : mc + 1],
                )

            # ---- emission helpers -----------------------------------------
            # Tile's scheduler orders per-engine streams by dependency +
            # emission priority, so emission order biases what the PE does
            # while waiting on DMA.

            def emit_batch_dmas(b, pre_tt0=None):
                hT_b = hT[b].rearrange("(kc p) t -> p kc t", p=P)
                ht_tiles = []
                for tt in range(NT):
                    if tt == 0 and pre_tt0 is not None:
                        ht_tiles.append(pre_tt0)
                        continue
                    htt = ht_pool.tile(
                        [P, KC, TT], BF16, tag="ht", name=f"ht_b{b}t{tt}"
                    )
                    nc.gpsimd.dma_start(
                        out=htt, in_=hT_b[:, :, tt * TT : (tt + 1) * TT]
                    )
                    ht_tiles.append(htt)
                mb_sb = mask_pool.tile([P, T], F32, tag="m", name=f"mb{b}")
                nc.sync.dma_start(out=mb_sb, in_=_bcast_part(maskf[b]))
                return ht_tiles, mb_sb

            def emit_mains(b, tt, ht_tiles):
                pps = []
                for mc in range(MC):
                    pp = ps_pool.tile(
                        [P, TT], F32, tag="ps", name=f"pp{b}_{tt}_{mc}"
                    )
                    for kc in range(KC):
                        nc.tensor.matmul(
                            pp,
                            lhsT=u_sb[:, kc, mc * P : (mc + 1) * P],
                            rhs=ht_tiles[tt][:, kc, :],
                            start=(kc == 0),
                            stop=(kc == KC - 1),
                        )
                    pps.append(pp)
                return pps

            def emit_tile_rest(b, tt, pps, ht_tiles, mb_sb, st):
                # tanh + v-dot, then the online-softmax tile pass:
                #   et  = (e + 512) * m   (masked -> 0; 512 > max|e| and
                #         exp(-512-max) underflows to exactly 0 in fp32,
                #         while ulp(512)=6.1e-5 keeps e's precision)
                #   nmax_i = -max(et); ex = exp(et - max_i); z_i = sum(ex)
                #   part[:, dc, i] = sum_t ex_t * hT[p, dc, t]
                nmax, zs, part, scr = st
                e_ps = e_pool.tile([P, TT], F32, tag="e", name=f"e{b}_{tt}")
                for mc in range(MC):
                    th = tanh_pool.tile(
                        [P, TT], BF16, tag="th", name=f"th{b}_{tt}_{mc}"
                    )
                    nc.scalar.activation(
                        out=th,
                        in_=pps[mc],
                        func=AF.Tanh,
                        bias=proj_sb[:, mc, b : b + 1],
                        scale=1.0,
                    )
                    nc.tensor.matmul(
                        e_ps,
                        lhsT=v_bc[:, mc, :],
                        rhs=th,
                        start=(mc == 0),
                        stop=(mc == MC - 1),
                    )
                et = p2_pool.tile([P, TT], F32, tag="et", name=f"et{b}_{tt}")
                nc.vector.scalar_tensor_tensor(
                    out=et,
                    in0=e_ps,
                    scalar=512.0,
                    in1=mb_sb[:, tt * TT : (tt + 1) * TT],
                    op0=ALU.add,
                    op1=ALU.mult,
                )
                nc.vector.tensor_reduce(
                    out=nmax[:, tt : tt + 1],
                    in_=et,
                    axis=mybir.AxisListType.X,
                    op=ALU.max,
                    negate=True,
                )
                ex = p2_pool.tile([P, TT], F32, tag="ex", name=f"ex{b}_{tt}")
                nc.scalar.activation(
                    out=ex,
                    in_=et,
                    func=AF.Exp,
                    bias=nmax[:, tt : tt + 1],
                    scale=1.0,
                    accum_out=zs[:, tt : tt + 1],
                )
                for dc in range(KC):
                    nc.vector.scalar_tensor_tensor(
                        out=scr,
                        in0=ht_tiles[tt][:, dc, :],
                        scalar=1.0,
                        in1=ex,
                        op0=ALU.mult,
                        op1=ALU.mult,
                        accum_out=part[:, dc, tt : tt + 1],
                    )

            def emit_batch_tail(b, st):
                # combine tiles: f_i = exp(max_i - M) with global max M,
                # ctx = sum_i part_i f_i / sum_i z_i f_i  (all tiny tiles)
                nmax, zs, part, scr = st
                negM = small_pool.tile([P, 1], F32, tag="negM", name=f"nM{b}")
                nc.vector.tensor_reduce(
                    out=negM, in_=nmax, axis=mybir.AxisListType.X, op=ALU.min
                )
                f = small_pool.tile([P, NT], F32, tag="f", name=f"f{b}")
                nc.scalar.activation(
                    out=f, in_=nmax, func=AF.Exp, bias=negM, scale=-1.0
                )
                fz = small_pool.tile([P, NT], F32, tag="fz", name=f"fz{b}")
                zf = small_pool.tile([P, 1], F32, tag="zf", name=f"zf{b}")
                nc.vector.scalar_tensor_tensor(
                    out=fz,
                    in0=zs,
                    scalar=1.0,
                    in1=f,
                    op0=ALU.mult,
                    op1=ALU.mult,
                    accum_out=zf,
                )
                sinv = small_pool.tile([P, 1], F32, tag="sinv", name=f"si{b}")
                nc.vector.reciprocal(sinv, zf)
                for tt in range(NT):
                    nc.vector.tensor_scalar_mul(
                        out=part[:, :, tt : tt + 1],
                        in0=part[:, :, tt : tt + 1],
                        scalar1=f[:, tt : tt + 1],
                    )
                ctx = ctx_pool.tile([P, KC], F32, tag="ctx", name=f"cx{b}")
                nc.vector.tensor_reduce(
                    out=ctx, in_=part, axis=mybir.AxisListType.X, op=ALU.add
                )
                nc.vector.tensor_scalar_mul(out=ctx, in0=ctx, scalar1=sinv)
                nc.scalar.dma_start(
                    out=out[b].rearrange("(dc p) -> p dc", p=P), in_=ctx
                )

            def batch_state(b):
                nmax = small_pool.tile([P, NT], F32, tag="nmax", name=f"nm{b}")
                zs = small_pool.tile([P, NT], F32, tag="zs", name=f"zs{b}")
                part = ctx_pool.tile([P, KC, NT], F32, tag="part", name=f"pt{b}")
                scr = p2_pool.tile([P, TT], F32, tag="scr", name=f"sc{b}")
                return nmax, zs, part, scr

            def emit_proj():
                # proj_s = s @ W (sT-stationary: the weight load is only
                # BL=4 columns), then 16 PE transposes of (4,128) chunks put
                # H on partitions for the tanh bias. No DRAM round-trip —
                # its tiny-line descriptors clogged a DMA queue for ~50us.
                pnat = []
                for i in range(2):
                    pn = e_pool.tile([BL, TT], F32, tag="e", name=f"pnat{i}")
                    pnat.append(pn)
                for kc in range(KC):
                    for nh in range(2):
                        nc.tensor.matmul(
                            pnat[nh],
                            lhsT=sT_sb[:, kc, :],
                            rhs=w_sb[:, kc, nh * TT : (nh + 1) * TT],
                            start=(kc == 0),
                            stop=(kc == KC - 1),
                        )
                pstg = singles.tile([BL, H], F32)
                for nh in range(2):
                    nc.vector.tensor_copy(
                        out=pstg[:, nh * TT : (nh + 1) * TT], in_=pnat[nh]
                    )
                proj_sb = singles.tile([P, MC, BL], F32)
                for mc in range(MC):
                    tp = e_pool.tile([P, BL], F32, tag="e", name=f"tp{mc}")
                    nc.tensor.transpose(
                        tp, in_=pstg[:, mc * P : (mc + 1) * P], identity=identity4
                    )
                    nc.vector.tensor_copy(out=proj_sb[:, mc, :], in_=tp)
                return proj_sb

            identity4 = singles.tile([BL, BL], F32)
            make_identity(nc, identity4)

            # ---- pipeline -------------------------------------------------
            # Batch 0, tile 0's main matmuls are emitted BEFORE proj: they
            # only need hT(b0,tt0) + the first U chunk, both of which land
            # well before all of W, so the PE warms up on dense main work
            # while W trickles in; the scheduler slots proj into the psum-
            # runway stall that follows.
            ht0, mb0 = emit_batch_dmas(0, pre_tt0=ht00)
            st0 = batch_state(0)
            pps00 = emit_mains(0, 0, ht0)
            proj_sb = emit_proj()
            emit_tile_rest(0, 0, pps00, ht0, mb0, st0)
            for tt in range(1, NT):
                pps = emit_mains(0, tt, ht0)
                emit_tile_rest(0, tt, pps, ht0, mb0, st0)
            emit_batch_tail(0, st0)

            for b in range(1, BL):
                ht_tiles, mb_sb = emit_batch_dmas(b)
                st = batch_state(b)
                for tt in range(NT):
                    pps = emit_mains(b, tt, ht_tiles)
                    emit_tile_rest(b, tt, pps, ht_tiles, mb_sb, st)
                emit_batch_tail(b, st)

    nc.compile()
    return nc


_NC_CACHE = None


def _get_module():
    global _NC_CACHE
    if _NC_CACHE is None:
        _NC_CACHE = build_module()
    return _NC_CACHE


def core_in_map(s, h, mask, W, U, v, c):
    """Shard + lay out the full inputs for core c."""
    bs = slice(c * BL, (c + 1) * BL)
    return {
        "hT": np.ascontiguousarray(
            np.asarray(h, np.float32)[bs]
            .transpose(0, 2, 1)
            .astype(ml_dtypes.bfloat16)
        ),
        "sT": np.ascontiguousarray(np.asarray(s, np.float32)[0, bs].T),
        "maskf": np.ascontiguousarray(np.asarray(mask)[bs].astype(np.float32)),
        "W": np.ascontiguousarray(np.asarray(W, np.float32)),
        "U": np.ascontiguousarray(np.asarray(U, np.float32).astype(ml_dtypes.bfloat16)),
        "v": np.ascontiguousarray(np.asarray(v, np.float32).reshape(H, 1)),
    }


def kernel(s, h, mask, W, U, v):
    in_maps = [core_in_map(s, h, mask, W, U, v, c) for c in range(NCORES)]
    nc = _get_module()
    res = run_bass_kernel_spmd(nc, in_maps, list(range(NCORES)))
    return np.concatenate([res.results[c]["out"] for c in range(NCORES)], axis=0)


# revision 46
# speedup vs baseline: 1.0706x; 1.0213x over previous
"""Trainium2 Bass kernel for masked additive (Bahdanau-style) attention.

Computes, for each batch b:
    ph    = h_b @ U                     (T, H)
    e     = tanh(ph + s_b @ W) @ v      (T,)
    e     = where(mask, e, -1e9)
    score = softmax(e)                  (T,)
    ctx   = sum_t score_t * h_b[t]      (D,)

Key observations baked into the kernel:
  * The mask never needs to be applied to h: masked positions get energy
    -1e9, exp underflows to exactly 0 in fp32, so their contribution to
    the context is exactly 0 either way.
  * The big matmul (h @ U) is computed transposed: ph^T tiles with H on
    partitions, so the per-batch bias (s_b @ W) is a per-partition scalar
    that fuses into the tanh activation for free.
  * e is produced broadcast across all 128 partitions (the v-dot matmul
    uses a stationary operand whose 128 columns are all v), so the
    softmax runs at full 128-lane width with no partition reductions.
  * The softmax + context run flash-style per T-tile (local max/sum +
    fused multiply-accumulate over the resident h^T tile on the vector
    engine, rescaled at the end), so no h tile is ever touched twice, no
    serial softmax tail exists, and h needs no second load from HBM.
  * h^T and U are fed to the PE in bf16 (full-rate, half the HBM
    traffic); everything downstream of the big matmul accumulates in
    fp32. proj_s = s @ W stays in fp32r (full-rate fp32 storage).

Sharding: pure data parallelism, 4 batches per core on 8 cores; no
collectives. Host-side prep only shards and re-lays-out inputs
(transpose of h, bf16 casts).

Measured on trn2 (8 cores): ~322 us HW exec, scale-relative absmax
error ~4.6e-3 vs the fp32 reference.
"""

import ml_dtypes
import numpy as np

import concourse.bass as bass
import concourse.tile as tile
from concourse import bacc, mybir
from concourse.bass_utils import run_bass_kernel_spmd
from concourse.masks import make_identity

F32 = mybir.dt.float32
F32R = mybir.dt.float32r
BF16 = mybir.dt.bfloat16

B, T, D, H = 32, 2048, 1024, 1024
NCORES = 8
BL = B // NCORES          # batches per core
P = 128                   # partitions
KC = D // P               # 8 contraction chunks
MC = H // P               # 8 output-row chunks
TT = 512                  # T tile (fp32 moving-operand max, one PSUM bank)
NT = T // TT              # 4 T tiles per batch
AF = mybir.ActivationFunctionType
ALU = mybir.AluOpType


def _bcast_part(ap, parts=P):
    """Broadcast a 1-partition AP across `parts` partitions (step 0)."""
    return bass.AP(tensor=ap.tensor, offset=ap.offset, ap=[[0, parts]] + list(ap.ap))


def build_module():
    nc = bacc.Bacc(
        "TRN2",
        target_bir_lowering=False,
        debug=False,
        enable_asserts=False,
        num_devices=NCORES,
    )

    hT = nc.dram_tensor("hT", [BL, D, T], BF16, kind="ExternalInput").ap()
    sT = nc.dram_tensor("sT", [D, BL], F32R, kind="ExternalInput").ap()
    maskf = nc.dram_tensor("maskf", [BL, T], F32, kind="ExternalInput").ap()
    W = nc.dram_tensor("W", [D, H], F32R, kind="ExternalInput").ap()
    U = nc.dram_tensor("U", [D, H], BF16, kind="ExternalInput").ap()
    v = nc.dram_tensor("v", [H, 1], F32, kind="ExternalInput").ap()
    out = nc.dram_tensor("out", [BL, D], F32, kind="ExternalOutput").ap()

    with tile.TileContext(nc) as tc:
        with (
            tc.tile_pool(name="singles", bufs=1) as singles,
            tc.tile_pool(name="ht", bufs=5) as ht_pool,
            tc.tile_pool(name="mask", bufs=1) as mask_pool,
            tc.tile_pool(name="tanh", bufs=3) as tanh_pool,
            tc.tile_pool(name="p2", bufs=2) as p2_pool,
            tc.tile_pool(name="small", bufs=4) as small_pool,
            tc.tile_pool(name="ctx", bufs=2) as ctx_pool,
            tc.tile_pool(name="ps", bufs=6, space="PSUM") as ps_pool,
            tc.tile_pool(name="eps", bufs=2, space="PSUM") as e_pool,
        ):
            # ---- persistent operands -------------------------------------
            # DMA queue assignment (each issuing engine owns a DGE ring and
            # a ring drains in issue order, so placement controls priority):
            # gpsimd carries sT + W first, then all hT; scalar carries U and
            # the outputs; sync carries v + the mask broadcasts.
            sT_sb = singles.tile([P, KC, BL], F32R)
            nc.gpsimd.dma_start(
                out=sT_sb, in_=sT.rearrange("(kc p) b -> p kc b", p=P)
            )

            # W fully resident in kc-pair chunks, issued FIRST on the gpsimd
            # queue (ahead of all hT traffic): queues drain in order, so W —
            # which gates proj -> tanh -> everything — wins the early DMA
            # engines. (Streaming W in small WAR-chained chunks starved
            # behind hT in DMA-engine arbitration — measured 65us stall.)
            # batch 0 tile 0's hT goes ahead of even W: it unblocks the
            # first 5 main-matmul groups, which is what the PE runs while
            # W arrives for proj.
            ht00 = ht_pool.tile([P, KC, TT], BF16, tag="ht", name="ht_b0t0")
            nc.gpsimd.dma_start(
                out=ht00,
                in_=hT[0].rearrange("(kc p) t -> p kc t", p=P)[:, :, 0:TT],
            )

            w_sb = singles.tile([P, KC, H], F32R)
            W_r = W.rearrange("(kc p) n -> p kc n", p=P)
            for wc in range(4):
                nc.gpsimd.dma_start(
                    out=w_sb[:, 2 * wc : 2 * wc + 2, :],
                    in_=W_r[:, 2 * wc : 2 * wc + 2, :],
                )

            # U chunked by H-column pairs: the first main matmul group only
            # needs U[:, :, :128], so it starts ~3 chunks earlier.
            u_sb = singles.tile([P, KC, H], BF16)
            U_r = U.rearrange("(kc p) n -> p kc n", p=P)
            for uc in range(4):
                nc.scalar.dma_start(
                    out=u_sb[:, :, uc * 256 : (uc + 1) * 256],
                    in_=U_r[:, :, uc * 256 : (uc + 1) * 256],
                )

            # v replicated into a (P, MC, P) stationary operand: for each
            # H-chunk mc, all 128 columns equal v[mc*128 + p].
            v_col = singles.tile([P, MC], F32)
            nc.sync.dma_start(out=v_col, in_=v.rearrange("(mc p) x -> p (mc x)", p=P))
            v_bc = singles.tile([P, MC, P], BF16)
            for mc in range(MC):
                nc.vector.memset(v_bc[:, mc, :], 0.0)
                nc.vector.tensor_scalar_add(
                    out=v_bc[:, mc, :],
                    in0=v_bc[:, mc, :],
                    scalar1=v_col[:, mc : mc + 1],
                )

            # ---- emission helpers -----------------------------------------
            # Tile's scheduler orders per-engine streams by dependency +
            # emission priority, so emission order biases what the PE does
            # while waiting on DMA.

            def emit_batch_dmas(b, pre_tt0=None):
                hT_b = hT[b].rearrange("(kc p) t -> p kc t", p=P)
                ht_tiles = []
                for tt in range(NT):
                    if tt == 0 and pre_tt0 is not None:
                        ht_tiles.append(pre_tt0)
                        continue
                    htt = ht_pool.tile(
                        [P, KC, TT], BF16, tag="ht", name=f"ht_b{b}t{tt}"
                    )
                    nc.gpsimd.dma_start(
                        out=htt, in_=hT_b[:, :, tt * TT : (tt + 1) * TT]
                    )
                    ht_tiles.append(htt)
                mb_sb = mask_pool.tile([P, T], F32, tag="m", name=f"mb{b}")
                nc.sync.dma_start(out=mb_sb, in_=_bcast_part(maskf[b]))
                return ht_tiles, mb_sb

            def emit_mains(b, tt, ht_tiles):
                pps = []
                for mc in range(MC):
                    pp = ps_pool.tile(
                        [P, TT], F32, tag="ps", name=f"pp{b}_{tt}_{mc}"
                    )
                    for kc in range(KC):
                        nc.tensor.matmul(
                            pp,
                            lhsT=u_sb[:, kc, mc * P : (mc + 1) * P],
                            rhs=ht_tiles[tt][:, kc, :],
                            start=(kc == 0),
                            stop=(kc == KC - 1),
                        )
                    pps.append(pp)
                return pps

            def emit_tile_rest(b, tt, pps, ht_tiles, mb_sb, st):
                # tanh + v-dot, then the online-softmax tile pass:
                #   et  = (e + 512) * m   (masked -> 0; 512 > max|e| and
                #         exp(-512-max) underflows to exactly 0 in fp32,
                #         while ulp(512)=6.1e-5 keeps e's precision)
                #   nmax_i = -max(et); ex = exp(et - max_i); z_i = sum(ex)
                #   part[:, dc, i] = sum_t ex_t * hT[p, dc, t]
                nmax, zs, part, scr = st
                e_ps = e_pool.tile([P, TT], F32, tag="e", name=f"e{b}_{tt}")
                for mc in range(MC):
                    th = tanh_pool.tile(
                        [P, TT], BF16, tag="th", name=f"th{b}_{tt}_{mc}"
                    )
                    nc.scalar.activation(
                        out=th,
                        in_=pps[mc],
                        func=AF.Tanh,
                        bias=proj_sb[:, mc, b : b + 1],
                        scale=1.0,
                    )
                    nc.tensor.matmul(
                        e_ps,
                        lhsT=v_bc[:, mc, :],
                        rhs=th,
                        start=(mc == 0),
                        stop=(mc == MC - 1),
                    )
                et = p2_pool.tile([P, TT], F32, tag="et", name=f"et{b}_{tt}")
                nc.vector.scalar_tensor_tensor(
                    out=et,
                    in0=e_ps,
                    scalar=512.0,
                    in1=mb_sb[:, tt * TT : (tt + 1) * TT],
                    op0=ALU.add,
                    op1=ALU.mult,
                )
                nc.vector.tensor_reduce(
                    out=nmax[:, tt : tt + 1],
                    in_=et,
                    axis=mybir.AxisListType.X,
                    op=ALU.max,
                    negate=True,
                )
                ex = p2_pool.tile([P, TT], F32, tag="ex", name=f"ex{b}_{tt}")
                nc.scalar.activation(
                    out=ex,
                    in_=et,
                    func=AF.Exp,
                    bias=nmax[:, tt : tt + 1],
                    scale=1.0,
                    accum_out=zs[:, tt : tt + 1],
                )
                for dc in range(KC):
                    nc.vector.scalar_tensor_tensor(
                        out=scr,
                        in0=ht_tiles[tt][:, dc, :],
                        scalar=1.0,
                        in1=ex,
                        op0=ALU.mult,
                        op1=ALU.mult,
                        accum_out=part[:, dc, tt : tt + 1],
                    )

            def emit_batch_tail(b, st):
                # combine tiles: f_i = exp(max_i - M) with global max M,
                # ctx = sum_i part_i f_i / sum_i z_i f_i  (all tiny tiles)
                nmax, zs, part, scr = st
                negM = small_pool.tile([P, 1], F32, tag="negM", name=f"nM{b}")
                nc.vector.tensor_reduce(
                    out=negM, in_=nmax, axis=mybir.AxisListType.X, op=ALU.min
                )
                f = small_pool.tile([P, NT], F32, tag="f", name=f"f{b}")
                nc.scalar.activation(
                    out=f, in_=nmax, func=AF.Exp, bias=negM, scale=-1.0
                )
                fz = small_pool.tile([P, NT], F32, tag="fz", name=f"fz{b}")
                zf = small_pool.tile([P, 1], F32, tag="zf", name=f"zf{b}")
                nc.vector.scalar_tensor_tensor(
                    out=fz,
                    in0=zs,
                    scalar=1.0,
                    in1=f,
                    op0=ALU.mult,
                    op1=ALU.mult,
                    accum_out=zf,
                )
                sinv = small_pool.tile([P, 1], F32, tag="sinv", name=f"si{b}")
                nc.vector.reciprocal(sinv, zf)
                for tt in range(NT):
                    nc.vector.tensor_scalar_mul(
                        out=part[:, :, tt : tt + 1],
                        in0=part[:, :, tt : tt + 1],
                        scalar1=f[:, tt : tt + 1],
                    )
                ctx = ctx_pool.tile([P, KC], F32, tag="ctx", name=f"cx{b}")
                nc.vector.tensor_reduce(
                    out=ctx, in_=part, axis=mybir.AxisListType.X, op=ALU.add
                )
                nc.vector.tensor_scalar_mul(out=ctx, in0=ctx, scalar1=sinv)
                nc.scalar.dma_start(
                    out=out[b].rearrange("(dc p) -> p dc", p=P), in_=ctx
                )

            def batch_state(b):
                nmax = small_pool.tile([P, NT], F32, tag="nmax", name=f"nm{b}")
                zs = small_pool.tile([P, NT], F32, tag="zs", name=f"zs{b}")
                part = ctx_pool.tile([P, KC, NT], F32, tag="part", name=f"pt{b}")
                scr = p2_pool.tile([P, TT], F32, tag="scr", name=f"sc{b}")
                return nmax, zs, part, scr

            def emit_proj():
                # proj_s = s @ W (sT-stationary: the weight load is only
                # BL=4 columns), then 16 PE transposes of (4,128) chunks put
                # H on partitions for the tanh bias. No DRAM round-trip —
                # its tiny-line descriptors clogged a DMA queue for ~50us.
                pnat = []
                for i in range(2):
                    pn = e_pool.tile([BL, TT], F32, tag="e", name=f"pnat{i}")
                    pnat.append(pn)
                for kc in range(KC):
                    for nh in range(2):
                        nc.tensor.matmul(
                            pnat[nh],
                            lhsT=sT_sb[:, kc, :],
                            rhs=w_sb[:, kc, nh * TT : (nh + 1) * TT],
                            start=(kc == 0),
                            stop=(kc == KC - 1),
                        )
                pstg = singles.tile([BL, H], F32)
                for nh in range(2):
                    nc.vector.tensor_copy(
                        out=pstg[:, nh * TT : (nh + 1) * TT], in_=pnat[nh]
                    )
                proj_sb = singles.tile([P, MC, BL], F32)
                for mc in range(MC):
                    tp = e_pool.tile([P, BL], F32, tag="e", name=f"tp{mc}")
                    nc.tensor.transpose(
                        tp, in_=pstg[:, mc * P : (mc + 1) * P], identity=identity4
                    )
                    nc.vector.tensor_copy(out=proj_sb[:, mc, :], in_=tp)
                return proj_sb

            identity4 = singles.tile([BL, BL], F32)
            make_identity(nc, identity4)

            # ---- pipeline -------------------------------------------------
            # Batch 0, tile 0's main matmuls are emitted BEFORE proj: they
            # only need hT(b0,tt0) + the first U chunk, both of which land
            # well before all of W, so the PE warms up on dense main work
            # while W trickles in; the scheduler slots proj into the psum-
            # runway stall that follows.
            ht0, mb0 = emit_batch_dmas(0, pre_tt0=ht00)
            st0 = batch_state(0)
            pps00 = emit_mains(0, 0, ht0)
            proj_sb = emit_proj()
            emit_tile_rest(0, 0, pps00, ht0, mb0, st0)
            for tt in range(1, NT):
                pps = emit_mains(0, tt, ht0)
                emit_tile_rest(0, tt, pps, ht0, mb0, st0)
            emit_batch_tail(0, st0)

            for b in range(1, BL):
                ht_tiles, mb_sb = emit_batch_dmas(b)
                st = batch_state(b)
                for tt in range(NT):
                    pps = emit_mains(b, tt, ht_tiles)
                    emit_tile_rest(b, tt, pps, ht_tiles, mb_sb, st)
                emit_batch_tail(b, st)

    nc.compile()
    return nc


_NC_CACHE = None


def _get_module():
    global _NC_CACHE
    if _NC_CACHE is None:
        _NC_CACHE = build_module()
    return _NC_CACHE


def core_in_map(s, h, mask, W, U, v, c):
    """Shard + lay out the full inputs for core c."""
    bs = slice(c * BL, (c + 1) * BL)
    return {
        "hT": np.ascontiguousarray(
            np.asarray(h, np.float32)[bs]
            .transpose(0, 2, 1)
            .astype(ml_dtypes.bfloat16)
        ),
        "sT": np.ascontiguousarray(np.asarray(s, np.float32)[0, bs].T),
        "maskf": np.ascontiguousarray(np.asarray(mask)[bs].astype(np.float32)),
        "W": np.ascontiguousarray(np.asarray(W, np.float32)),
        "U": np.ascontiguousarray(np.asarray(U, np.float32).astype(ml_dtypes.bfloat16)),
        "v": np.ascontiguousarray(np.asarray(v, np.float32).reshape(H, 1)),
    }


def kernel(s, h, mask, W, U, v):
    in_maps = [core_in_map(s, h, mask, W, U, v, c) for c in range(NCORES)]
    nc = _get_module()
    res = run_bass_kernel_spmd(nc, in_maps, list(range(NCORES)))
    return np.concatenate([res.results[c]["out"] for c in range(NCORES)], axis=0)
